# revision 20
# baseline (speedup 1.0000x reference)
"""Canny edge detector (nn_CannyDetector) — Trainium2 Bass kernel, 8 cores.

Sharding: spatial bands (core k owns rows [128k, 128k+128) of all 4 images;
the flat-index NMS gather couples images, so every core computes all 4).

v2 design (vs v1): the sobel 3-taps are FOLDED into the conv band matrices
(15-tap combined filters), so PE produces gx/gy directly and the DVE 3-tap +
PSUM-shuffle stages disappear.  Channel sums gxa/gya are computed by two
extra PE matmuls on device-summed hx/hy.  Row-shifted m copies (mu/md) are
SBUF->SBUF DMAs instead of PE band matmuls.  The 9 overlapping input chunks
per (b,c) arrive in ONE DMA via a hand-built strided access pattern.  Mask
algebra runs bf16 (exact on {0,1}); elementwise work is spread across
DVE / Act / Pool.

Per core, per row-window (110 + 18 rows):
  A : hx,hy = 15-tap horizontal filters (gauss*[1,0,-1]h, gauss*[1,2,1]h)
      via 9 chunk matmuls per filter (stride 114), border variants fix the
      blur-col zero-padding at image cols 0/1023.  [PE]
  PQ: gx = ([1,2,1]v*gauss)^T hx, gy = ([1,0,-1]v*gauss)^T hy (blur-row
      validity folded into bands); gxa/gya = same bands on channel-summed
      hx/hy.  [PE]
  C : m_b = sum_c sqrt(gx_c^2+gy_c^2) (masked); mu/md row shifts by DMA;
      cross-image NMS (same decode as v1); hysteresis via bf16 [1,1,1] band.
"""
import sys
import numpy as np

if "/opt/trn_rl_repo" not in sys.path:
    sys.path.insert(0, "/opt/trn_rl_repo")

# ---------------- geometry ----------------
B, C, H, W = 4, 3, 1024, 1024
NCORES = 8
BAND = H // NCORES              # 128 rows per core
HALO = 9
SLABR = BAND + 2 * HALO         # 146 input rows per core
WP = 1040                       # 7 left zeros + 1024 + 9 right zeros
CS = 114                        # chunk output width (128 - 14 halo)
NCHUNK = 9                      # 9*114 = 1026 output cols (2 junk)
WINS = [(0, 110), (110, 18)]    # (start, R) output row windows within band
DIRS = [(0, 1), (1, 1), (1, 0), (1, -1)]   # d_b for b = 0..3 (E, SE, S, SW)

_cache = {}


def _build():
    import concourse.bass as bass
    import concourse.tile as tile
    from concourse import bacc, mybir
    from contextlib import ExitStack

    F32 = mybir.dt.float32
    BF16 = mybir.dt.bfloat16
    AF = mybir.ActivationFunctionType
    OP = mybir.AluOpType

    nc = bacc.Bacc("TRN2", target_bir_lowering=False, debug=False,
                   num_devices=NCORES)
    xT = nc.dram_tensor("xT", [B * C, WP, SLABR], F32, kind="ExternalInput").ap()
    # horizontal bands [128, 3*114]: mid | first | last variants
    bAX = nc.dram_tensor("bAX", [128, 342], F32, kind="ExternalInput").ap()
    bAY = nc.dram_tensor("bAY", [128, 342], F32, kind="ExternalInput").ap()
    # vertical bands per window: P | Q packed side by side
    bPQ0 = nc.dram_tensor("bPQ0", [128, 228], F32, kind="ExternalInput").ap()
    bPQ1 = nc.dram_tensor("bPQ1", [36, 44], F32, kind="ExternalInput").ap()
    bC3 = nc.dram_tensor("bC3", [128, 128], F32, kind="ExternalInput").ap()
    aux = nc.dram_tensor("aux", [128, 8], F32, kind="ExternalInput").ap()
    out = nc.dram_tensor("out", [B, BAND, W], F32, kind="ExternalOutput").ap()

    with tile.TileContext(nc) as tc, ExitStack() as ctx:
        dve, gp, act = nc.vector, nc.gpsimd, nc.scalar

        consts = ctx.enter_context(tc.tile_pool(name="consts", bufs=1))
        xcp = ctx.enter_context(tc.tile_pool(name="xc", bufs=2))
        psa = ctx.enter_context(tc.tile_pool(name="psa", bufs=3, space="PSUM"))
        psgx = ctx.enter_context(tc.tile_pool(name="psgx", bufs=1,
                                              space="PSUM"))
        psc = ctx.enter_context(tc.tile_pool(name="psc", bufs=1, space="PSUM"))
        hbp = ctx.enter_context(tc.tile_pool(name="hbp", bufs=3))
        sump = ctx.enter_context(tc.tile_pool(name="sump", bufs=1))
        scr = ctx.enter_context(tc.tile_pool(name="scr", bufs=1))
        mmp = ctx.enter_context(tc.tile_pool(name="mmp", bufs=1))
        gxyp = ctx.enter_context(tc.tile_pool(name="gxyp", bufs=1))
        ded = ctx.enter_context(tc.tile_pool(name="ded", bufs=1))
        scb = ctx.enter_context(tc.tile_pool(name="scb", bufs=5))
        hyp = ctx.enter_context(tc.tile_pool(name="hyp", bufs=1))

        bAXt = consts.tile([128, 342], F32)
        nc.sync.dma_start(bAXt[:], bAX[:])
        bAYt = consts.tile([128, 342], F32)
        nc.sync.dma_start(bAYt[:], bAY[:])
        bPQ0t = consts.tile([128, 228], F32)
        nc.sync.dma_start(bPQ0t[:], bPQ0[:])
        bPQ1t = consts.tile([36, 44], F32)
        nc.sync.dma_start(bPQ1t[:], bPQ1[:])
        bC3t = consts.tile([128, 128], F32)
        nc.sync.dma_start(bC3t[:], bC3[:])
        auxt = consts.tile([128, 8], F32)
        nc.sync.dma_start(auxt[:], aux[:])
        bC3b = consts.tile([128, 128], BF16)
        act.copy(bC3b[:], bC3t[:])
        ones16 = consts.tile([128, 1024], BF16)
        gp.memset(ones16[:], 1.0)

        TAN1 = float(np.float32(np.tan(np.pi / 8)))
        TAN3 = float(np.float32(np.tan(3 * np.pi / 8)))

        for wi, (wst, R) in enumerate(WINS):
            R4 = R + 4
            Rin = R + 18
            mM = auxt[0:R4, 4 + wi:5 + wi]
            mT = auxt[0:R4, 6 + wi:7 + wi]
            LOW = auxt[0:R4, 0:1]
            HIGH = auxt[0:R4, 1:2]
            if wi == 0:
                bP = bPQ0t[0:Rin, 0:R4]
                bQ = bPQ0t[0:Rin, 114:114 + R4]
            else:
                bP = bPQ1t[0:Rin, 0:R4]
                bQ = bPQ1t[0:Rin, 22:22 + R4]

            m_t = [None] * B
            mu_t = [None] * B
            md_t = [None] * B
            msk_t = [None] * B

            for b in range(B):
                mt = mmp.tile([128, 1026], F32, tag=f"m{b}")
                gp.memset(mt[0:R4, 0:1], 0.0)
                gp.memset(mt[0:R4, 1025:1026], 0.0)
                sq_t = []
                hx_t = []
                hy_t = []
                for c in range(C):
                    ci = b * C + c
                    # ---- phase A: one strided DMA, 9 chunks x 2 filters ----
                    xc = xcp.tile([128, 9 * Rin], F32, tag="xc")
                    xc3 = xc.rearrange("p (j r) -> p j r", j=9)
                    src = type(xT)(
                        xT.tensor,
                        ci * WP * SLABR + wst,
                        [[SLABR, 128], [CS * SLABR, 9], [1, Rin]])
                    nc.sync.dma_start(xc3[:, :, :], src)
                    hxs = hbp.tile([128, 1026], F32, tag="hxs")
                    hys = hbp.tile([128, 1026], F32, tag="hys")
                    for f, bA, evac_eng, hout in (
                            (0, bAXt, act, hxs), (1, bAYt, act, hys)):
                        for g in range(3):          # chunk groups 0-3,4-7,8
                            cks = range(4 * g, min(4 * g + 4, NCHUNK))
                            pt = psa.tile([128, 512], F32, tag="psa")
                            for cc in cks:
                                if cc == 0:
                                    bv = bA[:, 114:228]
                                elif cc == NCHUNK - 1:
                                    bv = bA[:, 228:342]
                                else:
                                    bv = bA[:, 0:114]
                                nc.tensor.matmul(
                                    pt[0:Rin, CS * (cc % 4):CS * (cc % 4) + CS],
                                    xc3[:, cc, :], bv, start=True, stop=True)
                            n = len(cks) * CS
                            if evac_eng is act:
                                act.copy(hout[0:Rin, CS * 4 * g:CS * 4 * g + n],
                                         pt[0:Rin, 0:n])
                            else:
                                gp.tensor_copy(
                                    hout[0:Rin, CS * 4 * g:CS * 4 * g + n],
                                    pt[0:Rin, 0:n])
                    hx_t.append(hxs)
                    hy_t.append(hys)

                    # ---- PQ: vertical bands -> gx/gy ----
                    gxp = psgx.tile([128, 1024], F32, tag="gx")
                    gyp = psgx.tile([128, 1024], F32, tag="gy")
                    for h in (0, 1):
                        nc.tensor.matmul(gxp[0:R4, 512 * h:512 * h + 512],
                                         bP, hxs[0:Rin, 512 * h:512 * h + 512],
                                         start=True, stop=True)
                        nc.tensor.matmul(gyp[0:R4, 512 * h:512 * h + 512],
                                         bQ, hys[0:Rin, 512 * h:512 * h + 512],
                                         start=True, stop=True)
                    sx = scr.tile([128, 1024], F32, tag="sx")
                    act.activation(sx[0:R4, :], gxp[0:R4, :], AF.Square)
                    sy = scr.tile([128, 1024], F32, tag="sy")
                    act.activation(sy[0:R4, :], gyp[0:R4, :], AF.Square)
                    gp.tensor_tensor(sx[0:R4, :], sx[0:R4, :], sy[0:R4, :],
                                     OP.add)
                    sq = scr.tile([128, 1024], F32, tag=f"sq{c}")
                    act.activation(sq[0:R4, :], sx[0:R4, :], AF.Sqrt, scale=mM)
                    sq_t.append(sq)
                # m = (s0 + s1) + s2   (reference channel-sum order)
                s01 = scr.tile([128, 1024], F32, tag="s01")
                dve.tensor_tensor(s01[0:R4, :], sq_t[0][0:R4, :],
                                  sq_t[1][0:R4, :], OP.add)
                dve.tensor_tensor(mt[0:R4, 1:1025], s01[0:R4, :],
                                  sq_t[2][0:R4, :], OP.add)
                m_t[b] = mt

                # ---- mu/md row shifts via SBUF->SBUF DMA ----
                mu = mmp.tile([128, 1026], F32, tag=f"mu{b}")
                gp.memset(mu[0:R4, 0:1026], 0.0)
                nc.sync.dma_start(mu[0:R4 - 1, 0:1026], mt[1:R4, 0:1026])
                md = mmp.tile([128, 1026], F32, tag=f"md{b}")
                gp.memset(md[0:R4, 0:1026], 0.0)
                nc.sync.dma_start(md[1:R4, 0:1026], mt[0:R4 - 1, 0:1026])
                mu_t[b], md_t[b] = mu, md

                # ---- channel-summed hx/hy -> gxa/gya via PE ----
                hxsum = sump.tile([128, 1026], F32, tag="hxsum")
                dve.tensor_tensor(hxsum[0:Rin, :], hx_t[0][0:Rin, :],
                                  hx_t[1][0:Rin, :], OP.add)
                dve.tensor_tensor(hxsum[0:Rin, :], hxsum[0:Rin, :],
                                  hx_t[2][0:Rin, :], OP.add)
                hysum = sump.tile([128, 1026], F32, tag="hysum")
                gp.tensor_tensor(hysum[0:Rin, :], hy_t[0][0:Rin, :],
                                 hy_t[1][0:Rin, :], OP.add)
                gp.tensor_tensor(hysum[0:Rin, :], hysum[0:Rin, :],
                                 hy_t[2][0:Rin, :], OP.add)
                gxa = psgx.tile([128, 1024], F32, tag="gx")
                gya = psgx.tile([128, 1024], F32, tag="gy")
                for h in (0, 1):
                    nc.tensor.matmul(gxa[0:R4, 512 * h:512 * h + 512],
                                     bP, hxsum[0:Rin, 512 * h:512 * h + 512],
                                     start=True, stop=True)
                    nc.tensor.matmul(gya[0:R4, 512 * h:512 * h + 512],
                                     bQ, hysum[0:Rin, 512 * h:512 * h + 512],
                                     start=True, stop=True)

                # ---- orientation masks ----
                ax = gxyp.tile([128, 1024], F32, tag="ax")
                act.activation(ax[0:R4, :], gxa[0:R4, :], AF.Abs)
                ay = gxyp.tile([128, 1024], F32, tag="ay")
                act.activation(ay[0:R4, :], gya[0:R4, :], AF.Abs)
                gxs = gxyp.tile([128, 1024], F32, tag="gxs")
                dve.tensor_copy(gxs[0:R4, :], gxa[0:R4, :])
                sp = gxyp.tile([128, 1024], F32, tag="sp")
                dve.tensor_tensor(sp[0:R4, :], gxs[0:R4, :], gya[0:R4, :],
                                  OP.mult)
                c1 = ded.tile([128, 1024], BF16, tag=f"c1_{b}")
                dve.scalar_tensor_tensor(c1[0:R4, :], ax[0:R4, :], TAN1,
                                         ay[0:R4, :], OP.mult, OP.is_ge)
                c2 = ded.tile([128, 1024], BF16, tag=f"c2_{b}")
                dve.scalar_tensor_tensor(c2[0:R4, :], ax[0:R4, :], TAN3,
                                         ay[0:R4, :], OP.mult, OP.is_lt)
                pos = ded.tile([128, 1024], BF16, tag="pos")
                gp.tensor_scalar(pos[0:R4, :], sp[0:R4, :], 0.0, None,
                                 OP.is_gt)
                dg = ded.tile([128, 1024], BF16, tag="dg")
                dve.tensor_tensor(dg[0:R4, :], ones16[0:R4, :], c1[0:R4, :],
                                  OP.subtract)
                dve.tensor_tensor(dg[0:R4, :], dg[0:R4, :], c2[0:R4, :],
                                  OP.subtract)
                dp = ded.tile([128, 1024], BF16, tag=f"dp_{b}")
                dve.tensor_tensor(dp[0:R4, :], dg[0:R4, :], pos[0:R4, :],
                                  OP.mult)
                dn = ded.tile([128, 1024], BF16, tag=f"dn_{b}")
                dve.tensor_tensor(dn[0:R4, :], dg[0:R4, :], dp[0:R4, :],
                                  OP.subtract)
                msk_t[b] = (c1, c2, dp, dn)

            def shifted(i, dy, dx):
                src = {0: m_t, 1: mu_t, -1: md_t}[dy][i]
                return src[0:R4, 1 + dx:1 + dx + 1024]

            for b in range(B):
                c1, c2, dp, dn = msk_t[b]
                dy, dx = DIRS[b]
                im = hyp.tile([128, 1024], F32, tag="im")
                acc = None
                cmpi = 0
                for pi, (mask, J, sg) in enumerate(
                        [(c1, 0, 1), (c2, 1, 1), (dp, 0, -1), (dn, 1, -1)]):
                    pp = scb.tile([128, 1024], BF16, tag="scb")
                    pfirst = None
                    for i in (J, J + 2):
                        cmp_ = scb.tile([128, 1024], BF16, tag="scb")
                        eng = dve
                        cmpi += 1
                        eng.tensor_tensor(cmp_[0:R4, :],
                                          m_t[i][0:R4, 1:1025],
                                          shifted(i, sg * dy, sg * dx),
                                          OP.is_gt)
                        if pfirst is None:
                            pfirst = cmp_
                        else:
                            dve.tensor_tensor(pp[0:R4, :], pfirst[0:R4, :],
                                              cmp_[0:R4, :], OP.mult)
                    t_ = scb.tile([128, 1024], BF16, tag="scb")
                    dve.tensor_tensor(t_[0:R4, :], mask[0:R4, :], pp[0:R4, :],
                                      OP.mult)
                    if acc is None:
                        acc = t_
                    elif pi < 3:
                        a2 = scb.tile([128, 1024], BF16, tag="scb")
                        dve.tensor_tensor(a2[0:R4, :], acc[0:R4, :],
                                          t_[0:R4, :], OP.add)
                        acc = a2
                    else:
                        dve.tensor_tensor(im[0:R4, :], acc[0:R4, :],
                                          t_[0:R4, :], OP.add)
                thin = hyp.tile([128, 1024], F32, tag="thin")
                gp.tensor_tensor(thin[0:R4, :], im[0:R4, :],
                                 m_t[b][0:R4, 1:1025], OP.mult)

                # ---- hysteresis ----
                hp = hyp.tile([128, 1026], BF16, tag="hp")
                gp.memset(hp[0:R4, 0:1], 0.0)
                gp.memset(hp[0:R4, 1025:1026], 0.0)
                gp.tensor_scalar(hp[0:R4, 1:1025], thin[0:R4, :], HIGH, None,
                                 OP.is_gt)
                m1 = hyp.tile([128, 1024], F32, tag="im")
                gp.tensor_scalar(m1[0:R4, :], thin[0:R4, :], HIGH, None,
                                 OP.is_le)
                mid = hyp.tile([128, 1024], BF16, tag="mid")
                dve.scalar_tensor_tensor(mid[0:R4, :], thin[0:R4, :], LOW,
                                         m1[0:R4, :], OP.is_ge, OP.mult)
                r3 = hyp.tile([128, 1024], BF16, tag="r3")
                dve.tensor_tensor(r3[0:R4, :], hp[0:R4, 0:1024],
                                  hp[0:R4, 2:1026], OP.add)
                dve.tensor_tensor(r3[0:R4, :], r3[0:R4, :], hp[0:R4, 1:1025],
                                  OP.add)
                gate = hyp.tile([128, 1024], BF16, tag="gate")
                for h in (0, 1):
                    c3p = psc.tile([128, 512], F32, tag="c3")
                    nc.tensor.matmul(c3p[0:R4, 0:512], bC3b[0:R4, 0:R4],
                                     r3[0:R4, 512 * h:512 * h + 512],
                                     start=True, stop=True)
                    dve.tensor_scalar(gate[0:R4, 512 * h:512 * h + 512],
                                      c3p[0:R4, 0:512], 0.0, None, OP.is_gt)
                g_ = hyp.tile([128, 1024], BF16, tag="g_")
                dve.tensor_tensor(g_[0:R4, :], gate[0:R4, :], mid[0:R4, :],
                                  OP.mult)
                th = hyp.tile([128, 1024], BF16, tag="th")
                dve.tensor_tensor(th[0:R4, :], hp[0:R4, 1:1025], g_[0:R4, :],
                                  OP.max)
                thm = hyp.tile([128, 1024], F32, tag="thm")
                act.activation(thm[0:R4, :], th[0:R4, :], AF.Copy, scale=mT)
                gp.memset(thm[0:R4, 0:1], 0.0)
                gp.memset(thm[0:R4, 1023:1024], 0.0)
                nc.sync.dma_start(out[b, wst:wst + R, 0:1024],
                                  thm[2:2 + R, 0:1024])

    nc.compile()
    return nc


def _gh_at(gh, idx):
    idx = np.asarray(idx)
    return np.where((idx >= 0) & (idx <= 12),
                    gh[np.clip(idx, 0, 12)], 0.0).astype(np.float32)


def _h_bands(gh):
    """Horizontal 15-tap folded bands [128, 114] x {mid, first, last}."""
    def band(weights, kind):
        ba = np.zeros((128, CS), np.float32)
        p = np.arange(128)[:, None]
        n = np.arange(CS)[None, :]
        for u, wgt in weights:
            g = _gh_at(gh, p - n - u - 1)
            if kind == 'first':
                validu = (n + u >= 0)
            elif kind == 'last':
                # j = 114*8 + n; j+u must be < 1024 -> n+u <= 111; and the
                # junk output cols n >= 112 are zeroed entirely.
                validu = (n + u <= 111) & (n <= 111)
            else:
                validu = np.ones_like(n, bool)
            ba += wgt * g * validu
        return ba

    wx = ((-1, 1.0), (1, -1.0))
    wy = ((-1, 1.0), (0, 2.0), (1, 1.0))
    bAX = np.concatenate([band(wx, 'mid'), band(wx, 'first'),
                          band(wx, 'last')], axis=1)
    bAY = np.concatenate([band(wy, 'mid'), band(wy, 'first'),
                          band(wy, 'last')], axis=1)
    return bAX, bAY


def _v_bands(gh, g0, R):
    """Vertical 15-tap folded bands [Rin, R4] with blur-row validity."""
    R4, Rin = R + 4, R + 18
    p = np.arange(Rin)[:, None]
    i = np.arange(R4)[None, :]
    bP = np.zeros((Rin, R4), np.float32)
    bQ = np.zeros((Rin, R4), np.float32)
    for u, wp_, wq in ((-1, 1.0, 1.0), (0, 2.0, 0.0), (1, 1.0, -1.0)):
        valid = ((g0 - 2 + i + u >= 0) & (g0 - 2 + i + u < H)).astype(
            np.float32)
        g = _gh_at(gh, p - i - u - 1)
        bP += wp_ * valid * g
        bQ += wq * valid * g
    return bP, bQ


def _host_prep(img, gauss_h):
    gh = np.asarray(gauss_h, np.float32).reshape(-1)

    flat = img.reshape(-1)
    r = (flat.size - 1) // 2
    v = np.partition(flat, r)[r]
    t1 = np.float32(max(np.float32(0.0),
                        np.float32(np.float32(0.7) * v)) * np.float32(6.0))
    t2 = np.float32(min(np.float32(1.0),
                        np.float32(np.float32(1.3) * v)) * np.float32(6.0))
    low = np.float32(min(t1, t2))
    high = np.float32(max(t1, t2))

    bAX, bAY = _h_bands(gh)

    c111 = np.zeros((128, 128), np.float32)
    t5 = np.arange(128)[:, None] - np.arange(128)[None, :]
    c111[np.abs(t5) <= 1] = 1.0

    xpad = np.zeros((B, C, H + 2 * HALO, WP), np.float32)
    xpad[:, :, HALO:HALO + H, 7:7 + W] = img

    in_maps = []
    for k in range(NCORES):
        slab = xpad[:, :, BAND * k:BAND * k + SLABR, :]  # [B,C,SLABR,WP]
        xTk = np.ascontiguousarray(
            slab.reshape(B * C, SLABR, WP).transpose(0, 2, 1))
        aux = np.zeros((128, 8), np.float32)
        aux[:, 0] = low
        aux[:, 1] = high
        pq = []
        for wi, (wst, R) in enumerate(WINS):
            R4 = R + 4
            g0 = BAND * k + wst
            for i in range(R4):
                gr = g0 - 2 + i
                aux[i, 4 + wi] = 1.0 if 0 <= gr < H else 0.0
                aux[i, 6 + wi] = 0.0 if (gr <= 0 or gr >= H - 1) else 1.0
            pq.append(_v_bands(gh, g0, R))
        b0 = np.zeros((128, 228), np.float32)
        b0[:, 0:114] = pq[0][0]
        b0[:, 114:228] = pq[0][1]
        b1 = np.zeros((36, 44), np.float32)
        b1[:, 0:22] = pq[1][0]
        b1[:, 22:44] = pq[1][1]
        in_maps.append({"xT": xTk, "bAX": bAX, "bAY": bAY,
                        "bPQ0": b0, "bPQ1": b1, "bC3": c111, "aux": aux})
    return in_maps


def kernel(img, gauss_h, gauss_v, sobel_h, sobel_v, dir_f, conn_f):
    from concourse import bass_utils

    img = np.ascontiguousarray(np.asarray(img, np.float32))
    in_maps = _host_prep(img, gauss_h)

    if "nc" not in _cache:
        _cache["nc"] = _build()
    nc = _cache["nc"]

    res = bass_utils.run_bass_kernel_spmd(
        nc, in_maps, core_ids=list(range(NCORES)))
    outs = [res.results[k]["out"] for k in range(NCORES)]
    full = np.concatenate(outs, axis=1)          # [B, H, W]
    return full[:, None, :, :].astype(np.float32)
